# revision 1
# baseline (speedup 1.0000x reference)
"""Bass/Trainium2 kernel for nn_Attention (Bahdanau-style attention).

  w1e   = enc @ W1.T                      [B, N, H]
  w2h   = h0 @ W2.T + b2                  [B, H]
  u     = tanh(w1e + w2h[:, None, :])     [B, N, H]
  logits= u @ V                           [B, N, 1]
  att   = softmax(logits, axis=1)
  out   = att^T @ enc                     [B, IN1]

Sharding: pure data-parallel over batch B=128 across 8 cores (16 batches
each); W1/W2/V replicated. No collectives.

Per-core dataflow (layout: tokens on partitions, H on free dim):
  - main matmul: stationary = enc^T tile [128 IN1, 128 tok] (host
    pre-transposed, bf16 shipped as uint16), moving = W1^T [128 IN1,
    512 H]; K=IN1=256 -> 2 accumulating matmuls per token tile; a 3rd
    K=1 matmul with an all-ones stationary row folds c = W2 h0 + b2
    (computed on device, broadcast to 128 partitions via a DRAM bounce)
    into the same PSUM accumulation. The 4 c-fold matmuls of a 4-tile
    group are row-packed at tile_position rows {0,32,64,96} so they run
    concurrently on the PE array.
  - tanh on ScalarE (PSUM->SBUF, bf16 out), 2 token-tiles per instr.
  - V-dot: one pair-wide tensor_tensor mult (u * V via a stride-0
    middle-dim broadcast AP), mostly on GpSimd, then a free-dim
    add-reduce -> logits columns [128, 16]. Pure-DVE pairs use one 3D
    tensor_reduce ([128,2,512] -> [128,2]); ~24/256 reduces go to
    ScalarE Identity+accum_out so ScalarE and VectorE stay balanced.
    (tensor_tensor_reduce and tensor_scalar+accum_out are broken on
    this toolchain: device wedge / BIR verifier reject.)
  - exp on ScalarE -> e [128, 16] bf16 (no max-subtract: |logits| <=
    ||V||_1 ~= 18, exp fits fp32/bf16 fine).
  - final weighted sum on PE: stationary = e column [128, 1], moving =
    enc natural tile [128 tok, 257] where column 256 is all-ones ->
    psum [1, 257] accumulates both att^T@enc AND the softmax denominator.
  - normalize by 1/S on ScalarE, DMA out per batch row.
"""

import os
import sys

for _p in ("/opt/trn_rl_repo",):
    if _p not in sys.path and os.path.isdir(_p):
        sys.path.insert(0, _p)

from contextlib import ExitStack

import ml_dtypes
import numpy as np

import concourse.bass as bass
from concourse import bacc, mybir, tile

B, N, IN1, IN2, H = 128, 2048, 256, 512, 512
NCORES = 8
BC = B // NCORES            # 16 batches per core
TOK = BC * N                # 32768 tokens per core
TPB = N // 128              # 16 token tiles per batch
NPAIR = TPB // 2            # 8 tile-pairs per batch
ENC_NW = 272                # padded natural width (257 used, 32B-aligned rows)

F32 = mybir.dt.float32
BF16 = mybir.dt.bfloat16

LAST_RUNNER = None

_CACHED_NC = None


class Runner:
    """Compile-once SPMD runner (replicates run_bass_via_pjrt's multi-core
    path) that keeps the jitted callable + device-resident inputs so
    repeated executions can be wall-clocked without compile/transfer."""

    def __init__(self, nc, in_maps):
        import jax
        from jax.experimental.shard_map import shard_map
        from jax.sharding import Mesh, NamedSharding, PartitionSpec

        from concourse import bass2jax, mybir as _mybir

        bass2jax.install_neuronx_cc_hook()
        self.jax = jax

        if not nc.is_finalized():
            nc.finalize()

        partition_name = (nc.partition_id_tensor.name
                          if nc.partition_id_tensor else None)
        in_names, out_names, out_avals, zero_outs = [], [], [], []
        for alloc in nc.m.functions[0].allocations:
            if not isinstance(alloc, _mybir.MemoryLocationSet):
                continue
            name = alloc.memorylocations[0].name
            if alloc.kind == "ExternalInput":
                if name != partition_name:
                    in_names.append(name)
            elif alloc.kind == "ExternalOutput":
                shape = tuple(alloc.tensor_shape)
                dtype = _mybir.dt.np(alloc.dtype)
                out_names.append(name)
                out_avals.append(jax.core.ShapedArray(shape, dtype))
                zero_outs.append(np.zeros(shape, dtype))
        n_params = len(in_names)
        all_in_names = list(in_names) + list(out_names)
        if partition_name is not None:
            all_in_names.append(partition_name)
        self.out_names = out_names

        def _body(*args):
            operands = list(args)
            if partition_name is not None:
                operands.append(bass2jax.partition_id_tensor())
            outs = bass2jax._bass_exec_p.bind(
                *operands,
                out_avals=tuple(out_avals),
                in_names=tuple(all_in_names),
                out_names=tuple(out_names),
                lowering_input_output_aliases=(),
                sim_require_finite=True,
                sim_require_nnan=True,
                nc=nc,
            )
            return tuple(outs)

        n_cores = len(in_maps)
        devices = jax.devices()[:n_cores]
        mesh = Mesh(np.asarray(devices), ("core",))
        spec = PartitionSpec("core")
        self.n_cores = n_cores
        self.out_avals = out_avals
        self.sharded = jax.jit(
            shard_map(_body, mesh=mesh,
                      in_specs=(spec,) * (n_params + len(out_names)),
                      out_specs=(spec,) * len(out_names),
                      check_rep=False),
            keep_unused=True,
        )

        def _body_chain(k):
            # k sequential executions chained through the output buffers:
            # each call's outputs become the next call's pre-zeroed output
            # operands, forcing true sequential execution in one dispatch.
            def f(*args):
                ins, zouts = args[:n_params], list(args[n_params:])
                for _ in range(k):
                    zouts = list(_body(*ins, *zouts))
                return tuple(zouts)
            return f

        self._chain_cache = {}
        self._mesh, self._spec = mesh, spec
        self._n_params = n_params
        self._shard_map, self._jit = shard_map, jax.jit
        self._body_chain = _body_chain
        sharding = NamedSharding(mesh, spec)
        self.dev_in = [
            jax.device_put(
                np.concatenate([np.asarray(in_maps[c][nm])
                                for c in range(n_cores)], axis=0), sharding)
            for nm in in_names
        ]
        self.dev_zeros = [
            jax.device_put(
                np.zeros((n_cores * z.shape[0], *z.shape[1:]), z.dtype), sharding)
            for z in zero_outs
        ]

    def run(self):
        out = self.sharded(*self.dev_in, *self.dev_zeros)
        self.jax.block_until_ready(out)
        return out

    def run_chain(self, k):
        # k async dispatches of the same executable; PJRT serializes them
        # on the device stream, so wall(k) - wall(1) ~= (k-1) * exec_time
        # (neuronx_cc_hook rejects >1 bass_exec per jitted module, so a
        # true in-graph chain is not compilable).
        out = None
        for _ in range(k):
            out = self.sharded(*self.dev_in, *self.dev_zeros)
        self.jax.block_until_ready(out)
        return out

    def outputs(self, out_arrs):
        return [
            {nm: np.asarray(out_arrs[i]).reshape(
                self.n_cores, *self.out_avals[i].shape)[c]
             for i, nm in enumerate(self.out_names)}
            for c in range(self.n_cores)
        ]


def build_nc(bc=BC, tpb=TPB):
    tok = bc * tpb * 128
    npair = tpb // 2
    nc = bacc.Bacc(None, target_bir_lowering=False)

    # NOTE: native bfloat16 ExternalInputs are mangled by the axon/PJRT
    # transfer path (measured: garbage values, device wedge). Ship bf16
    # bits as uint16 and bitcast on-chip.
    U16 = mybir.dt.uint16
    encT = nc.dram_tensor("encT", [IN1, tok], U16, kind="ExternalInput")
    encN = nc.dram_tensor("encN", [tok, ENC_NW], U16, kind="ExternalInput")
    w1t = nc.dram_tensor("w1t", [IN1, H], U16, kind="ExternalInput")
    h0t = nc.dram_tensor("h0t", [IN2, bc], U16, kind="ExternalInput")
    w2ta = nc.dram_tensor("w2ta", [IN2 + 1, H], U16, kind="ExternalInput")
    vb = nc.dram_tensor("vb", [128, H], U16, kind="ExternalInput")
    out = nc.dram_tensor("out", [bc, IN1], F32, kind="ExternalOutput")

    Tanh = mybir.ActivationFunctionType.Tanh
    Exp = mybir.ActivationFunctionType.Exp
    Copy = mybir.ActivationFunctionType.Copy
    Alu = mybir.AluOpType

    with tile.TileContext(nc) as tc, ExitStack() as ctx:
        consts = ctx.enter_context(tc.tile_pool(name="consts", bufs=1))
        etp = ctx.enter_context(tc.tile_pool(name="etp", bufs=3))
        enp = ctx.enter_context(tc.tile_pool(name="enp", bufs=6))
        upool = ctx.enter_context(tc.tile_pool(name="upool", bufs=4))
        lpool = ctx.enter_context(tc.tile_pool(name="lpool", bufs=2))
        epool = ctx.enter_context(tc.tile_pool(name="epool", bufs=2))
        spool = ctx.enter_context(tc.tile_pool(name="spool", bufs=4))
        zpool = ctx.enter_context(tc.tile_pool(name="zpool", bufs=3, space="PSUM"))
        opool = ctx.enter_context(tc.tile_pool(name="opool", bufs=2, space="PSUM"))

        # ---------------- prologue: constants ----------------
        sb_w1t = consts.tile([128, 2, H], BF16)
        for k in range(2):
            nc.sync.dma_start(out=sb_w1t[:, k, :].bitcast(U16),
                              in_=w1t[k * 128:(k + 1) * 128, :])
        sb_vb = consts.tile([128, H], BF16)
        sb_ones = consts.tile([1, 128], BF16)
        nc.vector.memset(sb_ones, 1.0)

        sb_h0t = consts.tile([128, 4, bc], BF16)
        for k in range(4):
            nc.sync.dma_start(out=sb_h0t[:, k, :].bitcast(U16),
                              in_=h0t[k * 128:(k + 1) * 128, :])
        sb_w2ta = consts.tile([128, 5, H], BF16)
        for k in range(4):
            nc.sync.dma_start(out=sb_w2ta[:, k, :].bitcast(U16),
                              in_=w2ta[k * 128:(k + 1) * 128, :])
        nc.sync.dma_start(out=sb_w2ta[0:1, 4, :].bitcast(U16),
                          in_=w2ta[IN2:IN2 + 1, :])

        # c = h0 @ W2.T + b2  -> [16, 512] in PSUM
        psum_c = zpool.tile([bc, H], F32, tag="z")
        for k in range(4):
            nc.tensor.matmul(psum_c, sb_h0t[:, k, :], sb_w2ta[:, k, :],
                             start=(k == 0), stop=False)
        nc.tensor.matmul(psum_c, sb_ones[0:1, 0:bc], sb_w2ta[0:1, 4, :],
                         start=False, stop=True)
        sb_c16 = consts.tile([bc, H], BF16)
        nc.vector.tensor_copy(sb_c16, psum_c)
        # c rows -> DRAM bounce -> broadcast to all 128 partitions, so the
        # K=1 c-fold matmuls can be row-packed at tile_position rows
        # {0,32,64,96} (4 concurrent on the PE array).
        dpool = ctx.enter_context(tc.tile_pool(name="dpool", bufs=1, space="DRAM"))
        c_dram = dpool.tile([bc, H], BF16)
        nc.gpsimd.dma_start(out=c_dram[:, :], in_=sb_c16[:, :])
        crep = consts.tile([128, bc * H], BF16)
        c_flat0 = bass.AP(tensor=c_dram.tensor, offset=c_dram.offset,
                          ap=[[0, 128], [1, H]])
        nc.gpsimd.dma_start(out=crep[:, 0:H], in_=c_flat0)
        c_flat1 = bass.AP(tensor=c_dram.tensor, offset=c_dram.offset + H,
                          ap=[[0, 128], [1, (bc - 1) * H]])
        nc.gpsimd.dma_start(out=crep[:, H:], in_=c_flat1)
        sb_onesq = consts.tile([128, 128], BF16)
        nc.vector.memset(sb_onesq, 1.0)
        nc.sync.dma_start(out=sb_vb.bitcast(U16), in_=vb[:, :])

        # ---------------- main pipeline ----------------
        for b in range(bc):
            sb_logits = lpool.tile([128, tpb], F32, tag="logits")
            for g8 in range(tpb // 8):             # 8 token tiles per DMA group
                tok8 = (b * tpb + g8 * 8) * 128
                sb_et = etp.tile([128, 2, 1024], BF16, tag="et")
                for k in range(2):
                    nc.sync.dma_start(
                        out=sb_et[:, k, :].bitcast(U16),
                        in_=encT[k * 128:(k + 1) * 128, tok8:tok8 + 1024])
                for sub in range(2):                # 4-tile compute sub-groups
                    grp = g8 * 2 + sub
                    pz0 = zpool.tile([128, 1024], F32, tag="z")
                    pz1 = zpool.tile([128, 1024], F32, tag="z")
                    pz = [pz0, pz1]
                    for q in range(4):              # main matmuls, 4 tiles
                        zs = pz[q // 2][:, (q % 2) * 512:(q % 2 + 1) * 512]
                        qq = sub * 4 + q
                        for k in range(2):
                            nc.tensor.matmul(
                                zs, sb_et[:, k, qq * 128:(qq + 1) * 128],
                                sb_w1t[:, k, :], start=(k == 0), stop=False)
                    first_grp = (b == 0 and g8 == 0 and sub == 0)
                    for q in range(4):              # row-packed c-fold matmuls
                        zs = pz[q // 2][:, (q % 2) * 512:(q % 2 + 1) * 512]
                        if first_grp:
                            # first group reads c directly from sb_c16 row 0
                            # (partition 0), skipping the crep DMA-chain
                            # latency at kernel startup
                            nc.tensor.matmul(zs, sb_onesq[0:1, :],
                                             sb_c16[0:1, :],
                                             start=False, stop=True)
                        else:
                            nc.tensor.matmul(zs, sb_onesq[32 * q:32 * q + 1, :],
                                             crep[32 * q:32 * q + 1,
                                                  b * H:(b + 1) * H],
                                             start=False, stop=True,
                                             tile_position=(32 * q, 0))
                    for hz in range(2):
                        sb_u = upool.tile([128, 1024], BF16, tag="u")
                        nc.scalar.activation(sb_u, pz[hz], Tanh)
                        sb_prod = upool.tile([128, 2, 512], BF16, tag="prod")
                        t0 = grp * 4 + hz * 2
                        # one pair-wide V multiply (V_bcast repeated along free)
                        # last batch's multiplies on VectorE: the tail
                        # chain (tanh->mult->reduce->exp->finals) is serial,
                        # and gpsimd adds queue-hop latency there
                        mul_eng = (nc.vector if (b == bc - 1 and g8 == 1)
                                   else nc.gpsimd if (t0 // 2) % 4 != 3
                                   else nc.vector)
                        mul_eng.tensor_tensor(
                            out=sb_prod,
                            in0=sb_u.rearrange("p (j f) -> p j f", j=2),
                            in1=bass.AP(tensor=sb_vb.tensor,
                                        offset=sb_vb.offset,
                                        ap=[sb_vb.ap[0], [0, 2],
                                            sb_vb.ap[1]]),
                            op=Alu.mult)
                        # ScalarE takes tile 5 every batch and tile 11 on
                        # odd batches (~24/256 reduces) to balance ACT~DVE
                        act_tiles = {5} if b % 2 == 0 else {5, 11}
                        if (t0 in act_tiles) or (t0 + 1 in act_tiles):
                            # mixed pair: per-tile reduces (one on ScalarE)
                            for half in range(2):
                                t_idx = t0 + half
                                lg = sb_logits[:, t_idx:t_idx + 1]
                                if t_idx in act_tiles:
                                    junk = upool.tile([128, 512], BF16,
                                                      tag="junk")
                                    nc.scalar.activation(
                                        junk, sb_prod[:, half, :],
                                        mybir.ActivationFunctionType.Identity,
                                        accum_out=lg)
                                else:
                                    nc.vector.tensor_reduce(
                                        out=lg, in_=sb_prod[:, half, :],
                                        op=Alu.add, axis=mybir.AxisListType.X)
                        else:
                            # pure-DVE pair: one 3D reduce -> two logit cols
                            nc.vector.tensor_reduce(
                                out=sb_logits[:, t0:t0 + 2], in_=sb_prod,
                                op=Alu.add, axis=mybir.AxisListType.X)

            sb_e = epool.tile([128, tpb], BF16, tag="e")
            if b == bc - 1:
                # last batch: exp in two chunks so its final matmuls start
                # before the last logits are reduced (shortens the drain tail)
                half_t = tpb // 2
                nc.scalar.activation(sb_e[:, 0:half_t],
                                     sb_logits[:, 0:half_t], Exp)
                nc.scalar.activation(sb_e[:, half_t:],
                                     sb_logits[:, half_t:], Exp)
            else:
                nc.scalar.activation(sb_e, sb_logits, Exp)

            psum_o = opool.tile([1, 257], F32, tag="o")
            for sg in range(tpb // 4):
                s0 = b * tpb + sg * 4
                sb_en = enp.tile([128, 4, ENC_NW], BF16, tag="en")
                nc.sync.dma_start(
                    out=sb_en.bitcast(U16),
                    in_=encN[s0 * 128:(s0 + 4) * 128, :].rearrange(
                        "(j p) c -> p j c", p=128))
                for j in range(4):
                    s = sg * 4 + j
                    nc.tensor.matmul(psum_o, sb_e[:, s:s + 1],
                                     sb_en[:, j, 0:257],
                                     start=(s == 0), stop=(s == tpb - 1))
            rS = spool.tile([1, 1], F32, tag="rs")
            nc.vector.reciprocal(rS, psum_o[0:1, 256:257])
            sb_out = spool.tile([1, IN1], F32, tag="obuf")
            nc.vector.tensor_scalar_mul(sb_out, psum_o[0:1, 0:256], rS)
            nc.sync.dma_start(out=out[b:b + 1, :], in_=sb_out)

    return nc


def _to_bf16(x):
    """bf16 bits as uint16 (native bf16 inputs are mangled by the
    transfer path - see build_nc note)."""
    return np.ascontiguousarray(x.astype(ml_dtypes.bfloat16)).view(np.uint16)


def kernel(**inputs):
    global LAST_RUNNER, _CACHED_NC
    enc = np.asarray(inputs["enc_outputs"], dtype=np.float32)   # [B, N, IN1]
    h0 = np.asarray(inputs["h0"], dtype=np.float32)             # [B, IN2]
    W1 = np.asarray(inputs["W1"], dtype=np.float32)             # [H, IN1]
    W2 = np.asarray(inputs["W2"], dtype=np.float32)             # [H, IN2]
    b2 = np.asarray(inputs["b2"], dtype=np.float32)             # [H]
    V = np.asarray(inputs["V"], dtype=np.float32)               # [H, 1]

    w1t = _to_bf16(W1.T)                                        # [IN1, H]
    w2ta = _to_bf16(np.concatenate([W2.T, b2[None, :]], 0))     # [IN2+1, H]
    vb = _to_bf16(np.broadcast_to(V.reshape(1, H), (128, H)))   # [128, H]

    in_maps = []
    for c in range(NCORES):
        enc_c = enc[c * BC:(c + 1) * BC]                        # [16, N, IN1]
        flat = enc_c.reshape(TOK, IN1)
        encT = _to_bf16(np.ascontiguousarray(flat.T))           # [IN1, TOK]
        encN = np.zeros((TOK, ENC_NW), dtype=ml_dtypes.bfloat16)
        encN[:, :IN1] = flat.astype(ml_dtypes.bfloat16)
        encN[:, IN1] = 1.0
        encN = encN.view(np.uint16)
        h0t = _to_bf16(h0[c * BC:(c + 1) * BC].T)               # [IN2, 16]
        in_maps.append({
            "encT": encT, "encN": encN, "w1t": w1t,
            "h0t": h0t, "w2ta": w2ta, "vb": vb,
        })

    if _CACHED_NC is None:
        _CACHED_NC = build_nc()
    nc = _CACHED_NC

    runner = Runner(nc, in_maps)
    LAST_RUNNER = runner
    results = runner.outputs(runner.run())
    out = np.concatenate([results[i]["out"] for i in range(NCORES)], axis=0)
    return out.astype(np.float32)



# revision 7
# speedup vs baseline: 1.3865x; 1.3865x over previous
"""Bass/Trainium2 kernel for nn_Attention (Bahdanau-style attention).

  w1e   = enc @ W1.T                      [B, N, H]
  w2h   = h0 @ W2.T + b2                  [B, H]
  u     = tanh(w1e + w2h[:, None, :])     [B, N, H]
  logits= u @ V                           [B, N, 1]
  att   = softmax(logits, axis=1)
  out   = att^T @ enc                     [B, IN1]

Sharding: pure data-parallel over batch B=128 across 8 cores (16 batches
each); W1/W2/V replicated. No collectives.

Per-core dataflow (H-major main matmul, fp8 residual quantization):
  - main matmul runs H-on-partitions: psum z[128 Hchunk, 1024 tok] per
    (chunk, half-batch).  enc and W1 are quantized to fp8 as q1=e4m3(x),
    plus raw residuals q2=e4m3(enc-q1), p2=e5m2(W1-p1); z = q1@p1 +
    q2@p1 + q1@p2 (the q2@p2 term is negligible).  Each term is ONE
    DoubleRow matmul contracting K=256 via 2 fp8 planes (measured w1e
    rms err 1.6e-3, better than bf16's 2.4e-3).
  - c = W2 h0 + b2 is computed H-major on the PE at startup
    ([128 H, 16 batches] psum) and folded into tanh as the per-partition
    activation BIAS - no PE/DVE cost in the main loop.
  - tanh on ScalarE per (chunk, half-batch): [128, 1024] psum -> sbuf
    u bf16.
  - V-dot uses u as the STATIONARY operand ([128 H, 128 tok] tiles) and
    V[chunk] as the 1-column MOVING operand: out [128 tok, 1] psum
    columns accumulate K=H over 4 chunk-matmuls.  Output lands
    token-major ([128, 16] logits per batch) so exp is a tiny ScalarE op.
  - finals: stationary = enc natural tile (bf16) [128 tok, 128 IN1],
    moving = e column [128,1] -> psum [128, 1] per IN1-chunk; plus an
    all-ones stationary for the softmax denominator S.  All finals
    matmuls have free-size 1.
  - numerators + S are staged to SBUF and shipped out once; the final
    divide by S happens on host during unsharding.
"""

import os
import sys

for _p in ("/opt/trn_rl_repo",):
    if _p not in sys.path and os.path.isdir(_p):
        sys.path.insert(0, _p)

from contextlib import ExitStack

import ml_dtypes
import numpy as np

import concourse.bass as bass
from concourse import bacc, mybir, tile

B, N, IN1, IN2, H = 128, 2048, 256, 512, 512
NCORES = 8
BC = B // NCORES            # 16 batches per core
TOK = BC * N                # 32768 tokens per core
HB = 1024                   # tokens per half-batch block
NHB = TOK // HB             # 32 half-batch blocks per core

F32 = mybir.dt.float32
BF16 = mybir.dt.bfloat16
F8E4 = mybir.dt.float8e4
F8E5 = mybir.dt.float8e5
U16 = mybir.dt.uint16
U8 = mybir.dt.uint8

LAST_RUNNER = None

_CACHED_NC = None


class Runner:
    """Compile-once SPMD runner (replicates run_bass_via_pjrt's multi-core
    path) that keeps the jitted callable + device-resident inputs so
    repeated executions can be wall-clocked without compile/transfer."""

    def __init__(self, nc, in_maps):
        import jax
        from jax.experimental.shard_map import shard_map
        from jax.sharding import Mesh, NamedSharding, PartitionSpec

        from concourse import bass2jax, mybir as _mybir

        bass2jax.install_neuronx_cc_hook()
        self.jax = jax

        if not nc.is_finalized():
            nc.finalize()

        partition_name = (nc.partition_id_tensor.name
                          if nc.partition_id_tensor else None)
        in_names, out_names, out_avals, zero_outs = [], [], [], []
        for alloc in nc.m.functions[0].allocations:
            if not isinstance(alloc, _mybir.MemoryLocationSet):
                continue
            name = alloc.memorylocations[0].name
            if alloc.kind == "ExternalInput":
                if name != partition_name:
                    in_names.append(name)
            elif alloc.kind == "ExternalOutput":
                shape = tuple(alloc.tensor_shape)
                dtype = _mybir.dt.np(alloc.dtype)
                out_names.append(name)
                out_avals.append(jax.core.ShapedArray(shape, dtype))
                zero_outs.append(np.zeros(shape, dtype))
        n_params = len(in_names)
        all_in_names = list(in_names) + list(out_names)
        if partition_name is not None:
            all_in_names.append(partition_name)
        self.out_names = out_names

        def _body(*args):
            operands = list(args)
            if partition_name is not None:
                operands.append(bass2jax.partition_id_tensor())
            outs = bass2jax._bass_exec_p.bind(
                *operands,
                out_avals=tuple(out_avals),
                in_names=tuple(all_in_names),
                out_names=tuple(out_names),
                lowering_input_output_aliases=(),
                sim_require_finite=True,
                sim_require_nnan=True,
                nc=nc,
            )
            return tuple(outs)

        n_cores = len(in_maps)
        devices = jax.devices()[:n_cores]
        mesh = Mesh(np.asarray(devices), ("core",))
        spec = PartitionSpec("core")
        self.n_cores = n_cores
        self.out_avals = out_avals
        self.sharded = jax.jit(
            shard_map(_body, mesh=mesh,
                      in_specs=(spec,) * (n_params + len(out_names)),
                      out_specs=(spec,) * len(out_names),
                      check_rep=False),
            keep_unused=True,
        )

        self._n_params = n_params
        sharding = NamedSharding(mesh, spec)
        self.dev_in = [
            jax.device_put(
                np.concatenate([np.asarray(in_maps[c][nm])
                                for c in range(n_cores)], axis=0), sharding)
            for nm in in_names
        ]
        self.dev_zeros = [
            jax.device_put(
                np.zeros((n_cores * z.shape[0], *z.shape[1:]), z.dtype), sharding)
            for z in zero_outs
        ]

    def run(self):
        out = self.sharded(*self.dev_in, *self.dev_zeros)
        self.jax.block_until_ready(out)
        return out

    def run_chain(self, k):
        # k async dispatches of the same executable; PJRT serializes them
        # on the device stream.
        out = None
        for _ in range(k):
            out = self.sharded(*self.dev_in, *self.dev_zeros)
        self.jax.block_until_ready(out)
        return out

    def outputs(self, out_arrs):
        return [
            {nm: np.asarray(out_arrs[i]).reshape(
                self.n_cores, *self.out_avals[i].shape)[c]
             for i, nm in enumerate(self.out_names)}
            for c in range(self.n_cores)
        ]


def build_nc(bc=BC):
    tok = bc * N
    nhb = tok // HB
    nc = bacc.Bacc(None, target_bir_lowering=False)

    # NOTE: native bf16/fp8 ExternalInputs are mangled by the axon/PJRT
    # transfer path; ship raw bits as uint16/uint8 and bitcast on-chip.
    q1d = nc.dram_tensor("q1d", [128, 2 * tok], U8, kind="ExternalInput")
    q2d = nc.dram_tensor("q2d", [128, 2 * tok], U8, kind="ExternalInput")
    w1p1 = nc.dram_tensor("w1p1", [128, 2 * 4 * 128], U8, kind="ExternalInput")
    w1p2 = nc.dram_tensor("w1p2", [128, 2 * 4 * 128], U8, kind="ExternalInput")
    encn = nc.dram_tensor("encn", [tok, IN1], U16, kind="ExternalInput")
    w2t = nc.dram_tensor("w2t", [IN2, H], U16, kind="ExternalInput")
    h0t = nc.dram_tensor("h0t", [IN2, bc], U16, kind="ExternalInput")
    b2r = nc.dram_tensor("b2r", [1, H], U16, kind="ExternalInput")
    vbr = nc.dram_tensor("vbr", [128, 4], U16, kind="ExternalInput")
    onum = nc.dram_tensor("onum", [128, 3 * bc], F32, kind="ExternalOutput")

    Tanh = mybir.ActivationFunctionType.Tanh
    Exp = mybir.ActivationFunctionType.Exp
    DR = mybir.MatmulPerfMode.DoubleRow

    with tile.TileContext(nc) as tc, ExitStack() as ctx:
        consts = ctx.enter_context(tc.tile_pool(name="consts", bufs=1))
        qpool = ctx.enter_context(tc.tile_pool(name="qpool", bufs=2))
        upool = ctx.enter_context(tc.tile_pool(name="upool", bufs=2))
        enp = ctx.enter_context(tc.tile_pool(name="enp", bufs=2))
        epool = ctx.enter_context(tc.tile_pool(name="epool", bufs=2))
        opool = ctx.enter_context(tc.tile_pool(name="opool", bufs=1))
        zpool = ctx.enter_context(tc.tile_pool(name="zpool", bufs=3, space="PSUM"))
        lpool = ctx.enter_context(tc.tile_pool(name="lpool", bufs=2, space="PSUM"))

        # ---------------- prologue: constants ----------------
        w1a = consts.tile([128, 2, 4, 128], F8E4)
        nc.sync.dma_start(out=w1a.bitcast(U8), in_=w1p1[:, :])
        w1b = consts.tile([128, 2, 4, 128], F8E5)
        nc.sync.dma_start(out=w1b.bitcast(U8), in_=w1p2[:, :])
        vbt = consts.tile([128, 4], BF16)
        nc.sync.dma_start(out=vbt.bitcast(U16), in_=vbr[:, :])
        w2s = consts.tile([128, 4, H], BF16)
        for k in range(4):
            nc.sync.dma_start(out=w2s[:, k, :].bitcast(U16),
                              in_=w2t[k * 128:(k + 1) * 128, :])
        h0s = consts.tile([128, 4, bc], BF16)
        for k in range(4):
            nc.sync.dma_start(out=h0s[:, k, :].bitcast(U16),
                              in_=h0t[k * 128:(k + 1) * 128, :])
        b2s = consts.tile([1, H], BF16)
        nc.sync.dma_start(out=b2s.bitcast(U16), in_=b2r[:, :])
        ones_col = consts.tile([128, 1], BF16)
        nc.vector.memset(ones_col, 1.0)
        ones_row = consts.tile([1, bc], BF16)
        nc.vector.memset(ones_row, 1.0)

        # c = (W2 h0 + b2), H-major: [128 H-in-chunk, chunk, batch]
        # (borrows an lpool slot so PSUM stays within 8 banks)
        psum_c = lpool.tile([128, 64], F32, tag="lt")
        for j in range(4):
            cs = psum_c[:, j * bc:(j + 1) * bc]
            for k in range(4):
                nc.tensor.matmul(cs, w2s[:, k, j * 128:(j + 1) * 128],
                                 h0s[:, k, :], start=(k == 0), stop=False)
            nc.tensor.matmul(cs, b2s[0:1, j * 128:(j + 1) * 128],
                             ones_row, start=False, stop=True)
        c_sb = consts.tile([128, 4, bc], F32)
        nc.vector.tensor_copy(c_sb, psum_c.rearrange("p (j b) -> p j b", j=4))

        osb = opool.tile([128, 3, bc], F32)

        # ---------------- main pipeline ----------------
        for b in range(bc):
            # logits + finals psum for this batch:
            #   cols 0..15  logits (token-major, tile s)
            #   cols 16,17  output numerator IN1-chunks
            #   col  18     softmax denominator S (partition 0)
            psum_lt = lpool.tile([128, 64], F32, tag="lt")
            en_sb = enp.tile([128, N // 128, IN1], BF16, tag="en")
            nc.gpsimd.dma_start(
                out=en_sb.bitcast(U16),
                in_=encn[b * N:(b + 1) * N, :].rearrange(
                    "(s p) c -> p s c", p=128))
            for hh in range(2):
                hb = 2 * b + hh
                t0 = hb * HB
                q1_sb = qpool.tile([128, 2, HB], F8E4, tag="q1")
                q2_sb = qpool.tile([128, 2, HB], F8E4, tag="q2")
                nc.sync.dma_start(
                    out=q1_sb.bitcast(U8),
                    in_=bass.AP(tensor=q1d, offset=t0,
                                ap=[[2 * tok, 128], [tok, 2], [1, HB]]))
                nc.sync.dma_start(
                    out=q2_sb.bitcast(U8),
                    in_=bass.AP(tensor=q2d, offset=t0,
                                ap=[[2 * tok, 128], [tok, 2], [1, HB]]))
                u_sb = upool.tile([128, 4, HB], BF16, tag="u")
                for j in range(4):
                    pz = zpool.tile([128, HB], F32, tag="z")
                    for v in range(HB // 512):   # psum-bank-sized outputs
                        pzv = pz[:, v * 512:(v + 1) * 512]
                        qs = slice(v * 512, (v + 1) * 512)
                        nc.tensor.matmul(pzv, w1a[:, :, j, :], q1_sb[:, :, qs],
                                         start=True, stop=False, perf_mode=DR)
                        nc.tensor.matmul(pzv, w1b[:, :, j, :], q1_sb[:, :, qs],
                                         start=False, stop=False, perf_mode=DR)
                        nc.tensor.matmul(pzv, w1a[:, :, j, :], q2_sb[:, :, qs],
                                         start=False, stop=True, perf_mode=DR)
                    nc.scalar.activation(u_sb[:, j, :], pz, Tanh,
                                         bias=c_sb[:, j, b:b + 1])
                for st in range(HB // 128):   # V-dot, 8 token tiles
                    s = hh * (HB // 128) + st
                    for j in range(4):
                        nc.tensor.matmul(
                            psum_lt[:, s:s + 1],
                            u_sb[:, j, st * 128:(st + 1) * 128],
                            vbt[:, j:j + 1],
                            start=(j == 0), stop=(j == 3))
            e_sb = epool.tile([128, bc], BF16, tag="e")
            nc.scalar.activation(e_sb, psum_lt[:, 0:bc], Exp)
            for s in range(N // 128):         # finals, 16 token tiles
                st, sp = (s == 0), (s == N // 128 - 1)
                ecol = e_sb[:, s:s + 1]
                nc.tensor.matmul(psum_lt[:, 16:17], en_sb[:, s, 0:128],
                                 ecol, start=st, stop=sp)
                nc.tensor.matmul(psum_lt[:, 17:18], en_sb[:, s, 128:256],
                                 ecol, start=st, stop=sp)
                nc.tensor.matmul(psum_lt[0:1, 18:19], ones_col,
                                 ecol, start=st, stop=sp)
            nc.vector.tensor_copy(osb[:, :, b], psum_lt[:, 16:19])

        nc.sync.dma_start(out=onum[:, :],
                          in_=osb.rearrange("p c b -> p (c b)"))

    return nc


def _bits16(x):
    return np.ascontiguousarray(x.astype(ml_dtypes.bfloat16)).view(np.uint16)


def kernel(**inputs):
    global LAST_RUNNER, _CACHED_NC
    enc = np.asarray(inputs["enc_outputs"], dtype=np.float32)   # [B, N, IN1]
    h0 = np.asarray(inputs["h0"], dtype=np.float32)             # [B, IN2]
    W1 = np.asarray(inputs["W1"], dtype=np.float32)             # [H, IN1]
    W2 = np.asarray(inputs["W2"], dtype=np.float32)             # [H, IN2]
    b2 = np.asarray(inputs["b2"], dtype=np.float32)             # [H]
    V = np.asarray(inputs["V"], dtype=np.float32)               # [H, 1]

    E4, E5 = ml_dtypes.float8_e4m3, ml_dtypes.float8_e5m2

    # W1 fp8 + residual, DoubleRow layout [p, plane, chunk, h]:
    # value = W1[chunk*128 + h, plane*128 + p]
    p1 = W1.astype(E4)
    p2 = (W1 - p1.astype(np.float32)).astype(E5)
    def w1_dr(q):
        # [H, IN1] -> [IN1-part 128, plane 2, chunk 4, h 128]
        a = q.reshape(4, 128, 2, 128).transpose(3, 2, 0, 1)
        return np.ascontiguousarray(a).view(np.uint8).reshape(128, 2 * 4 * 128)
    w1p1 = w1_dr(p1)
    w1p2 = w1_dr(p2)

    w2t = _bits16(W2.T)                                         # [IN2, H]
    b2r = _bits16(b2.reshape(1, H))
    vbr = _bits16(V.reshape(4, 128).T)                          # [128, 4]

    in_maps = []
    for c in range(NCORES):
        enc_c = enc[c * BC:(c + 1) * BC].reshape(TOK, IN1)      # [tok, 256]
        q1 = enc_c.astype(E4)
        q2 = (enc_c - q1.astype(np.float32)).astype(E4)
        def enc_dr(q):
            # [tok, IN1] -> [p 128, plane 2, tok]
            a = q.view(np.uint8).T.reshape(2, 128, TOK).transpose(1, 0, 2)
            return np.ascontiguousarray(a).reshape(128, 2 * TOK)
        h0t = _bits16(h0[c * BC:(c + 1) * BC].T)                # [IN2, 16]
        in_maps.append({
            "q1d": enc_dr(q1), "q2d": enc_dr(q2),
            "w1p1": w1p1, "w1p2": w1p2,
            "encn": _bits16(enc_c), "w2t": w2t, "h0t": h0t,
            "b2r": b2r, "vbr": vbr,
        })

    if _CACHED_NC is None:
        _CACHED_NC = build_nc()
    nc = _CACHED_NC

    runner = Runner(nc, in_maps)
    LAST_RUNNER = runner
    results = runner.outputs(runner.run())
    outs = []
    for c in range(NCORES):
        onum = results[c]["onum"].reshape(128, 3, BC)
        num = onum[:, 0:2, :].transpose(2, 1, 0).reshape(BC, IN1)
        s = onum[0, 2, :]                                       # [bc]
        outs.append(num / s[:, None])
    return np.concatenate(outs, axis=0).astype(np.float32)


# revision 15
# speedup vs baseline: 1.3865x; 1.0000x over previous
"""Bass/Trainium2 kernel for nn_Attention (Bahdanau-style attention).

  w1e   = enc @ W1.T                      [B, N, H]
  w2h   = h0 @ W2.T + b2                  [B, H]
  u     = tanh(w1e + w2h[:, None, :])     [B, N, H]
  logits= u @ V                           [B, N, 1]
  att   = softmax(logits, axis=1)
  out   = att^T @ enc                     [B, IN1]

Sharding: pure data-parallel over batch B=128 across 8 cores (16 batches
each); W1/W2/V replicated. No collectives.

Per-core dataflow (H-major main matmul, fp8 residual quantization):
  - main matmul runs H-on-partitions: psum z[128 Hchunk, 1024 tok] per
    (chunk, half-batch).  enc and W1 are quantized to fp8 as q1=e4m3(x),
    plus raw residuals q2=e4m3(enc-q1), p2=e5m2(W1-p1); z = q1@p1 +
    q2@p1 + q1@p2 (the q2@p2 term is negligible).  Each term is ONE
    DoubleRow matmul contracting K=256 via 2 fp8 planes (measured w1e
    rms err 1.6e-3, better than bf16's 2.4e-3).
  - c = W2 h0 + b2 is computed H-major on the PE at startup
    ([128 H, 16 batches] psum) and folded into tanh as the per-partition
    activation BIAS - no PE/DVE cost in the main loop.
  - tanh on ScalarE per (chunk, half-batch): [128, 1024] psum -> sbuf
    u bf16.
  - V-dot uses u as the STATIONARY operand ([128 H, 128 tok] tiles) and
    V[chunk] as the 1-column MOVING operand: out [128 tok, 1] psum
    columns accumulate K=H over 4 chunk-matmuls.  Output lands
    token-major ([128, 16] logits per batch) so exp is a tiny ScalarE op.
  - finals: stationary = enc natural tile (bf16) [128 tok, 128 IN1],
    moving = e column [128,1] -> psum [128, 1] per IN1-chunk; plus an
    all-ones stationary for the softmax denominator S.  All finals
    matmuls have free-size 1.
  - numerators + S are staged to SBUF and shipped out once; the final
    divide by S happens on host during unsharding.
"""

import os
import sys

for _p in ("/opt/trn_rl_repo",):
    if _p not in sys.path and os.path.isdir(_p):
        sys.path.insert(0, _p)

from contextlib import ExitStack

import ml_dtypes
import numpy as np

import concourse.bass as bass
from concourse import bacc, mybir, tile

B, N, IN1, IN2, H = 128, 2048, 256, 512, 512
NCORES = 8
BC = B // NCORES            # 16 batches per core
TOK = BC * N                # 32768 tokens per core
HB = 1024                   # tokens per half-batch block
NHB = TOK // HB             # 32 half-batch blocks per core

F32 = mybir.dt.float32
BF16 = mybir.dt.bfloat16
F8E4 = mybir.dt.float8e4
F8E5 = mybir.dt.float8e5
U16 = mybir.dt.uint16
U8 = mybir.dt.uint8

LAST_RUNNER = None

_CACHED_NC = None


class Runner:
    """Compile-once SPMD runner (replicates run_bass_via_pjrt's multi-core
    path) that keeps the jitted callable + device-resident inputs so
    repeated executions can be wall-clocked without compile/transfer."""

    def __init__(self, nc, in_maps):
        import jax
        from jax.experimental.shard_map import shard_map
        from jax.sharding import Mesh, NamedSharding, PartitionSpec

        from concourse import bass2jax, mybir as _mybir

        bass2jax.install_neuronx_cc_hook()
        self.jax = jax

        if not nc.is_finalized():
            nc.finalize()

        partition_name = (nc.partition_id_tensor.name
                          if nc.partition_id_tensor else None)
        in_names, out_names, out_avals, zero_outs = [], [], [], []
        for alloc in nc.m.functions[0].allocations:
            if not isinstance(alloc, _mybir.MemoryLocationSet):
                continue
            name = alloc.memorylocations[0].name
            if alloc.kind == "ExternalInput":
                if name != partition_name:
                    in_names.append(name)
            elif alloc.kind == "ExternalOutput":
                shape = tuple(alloc.tensor_shape)
                dtype = _mybir.dt.np(alloc.dtype)
                out_names.append(name)
                out_avals.append(jax.core.ShapedArray(shape, dtype))
                zero_outs.append(np.zeros(shape, dtype))
        n_params = len(in_names)
        all_in_names = list(in_names) + list(out_names)
        if partition_name is not None:
            all_in_names.append(partition_name)
        self.out_names = out_names

        def _body(*args):
            operands = list(args)
            if partition_name is not None:
                operands.append(bass2jax.partition_id_tensor())
            outs = bass2jax._bass_exec_p.bind(
                *operands,
                out_avals=tuple(out_avals),
                in_names=tuple(all_in_names),
                out_names=tuple(out_names),
                lowering_input_output_aliases=(),
                sim_require_finite=True,
                sim_require_nnan=True,
                nc=nc,
            )
            return tuple(outs)

        n_cores = len(in_maps)
        devices = jax.devices()[:n_cores]
        mesh = Mesh(np.asarray(devices), ("core",))
        spec = PartitionSpec("core")
        self.n_cores = n_cores
        self.out_avals = out_avals
        self.sharded = jax.jit(
            shard_map(_body, mesh=mesh,
                      in_specs=(spec,) * (n_params + len(out_names)),
                      out_specs=(spec,) * len(out_names),
                      check_rep=False),
            keep_unused=True,
        )

        self._n_params = n_params
        sharding = NamedSharding(mesh, spec)
        self.dev_in = [
            jax.device_put(
                np.concatenate([np.asarray(in_maps[c][nm])
                                for c in range(n_cores)], axis=0), sharding)
            for nm in in_names
        ]
        self.dev_zeros = [
            jax.device_put(
                np.zeros((n_cores * z.shape[0], *z.shape[1:]), z.dtype), sharding)
            for z in zero_outs
        ]

    def run(self):
        out = self.sharded(*self.dev_in, *self.dev_zeros)
        self.jax.block_until_ready(out)
        return out

    def run_chain(self, k):
        # k async dispatches of the same executable; PJRT serializes them
        # on the device stream.
        out = None
        for _ in range(k):
            out = self.sharded(*self.dev_in, *self.dev_zeros)
        self.jax.block_until_ready(out)
        return out

    def outputs(self, out_arrs):
        return [
            {nm: np.asarray(out_arrs[i]).reshape(
                self.n_cores, *self.out_avals[i].shape)[c]
             for i, nm in enumerate(self.out_names)}
            for c in range(self.n_cores)
        ]


def build_nc(bc=BC):
    tok = bc * N
    nhb = tok // HB
    nc = bacc.Bacc(None, target_bir_lowering=False)

    # NOTE: native bf16/fp8 ExternalInputs are mangled by the axon/PJRT
    # transfer path; ship raw bits as uint16/uint8 and bitcast on-chip.
    q1d = nc.dram_tensor("q1d", [128, 2 * tok], U8, kind="ExternalInput")
    q2d = nc.dram_tensor("q2d", [128, 2 * tok], U8, kind="ExternalInput")
    w1p1 = nc.dram_tensor("w1p1", [128, 2 * 4 * 128], U8, kind="ExternalInput")
    w1p2 = nc.dram_tensor("w1p2", [128, 2 * 4 * 128], U8, kind="ExternalInput")
    encn = nc.dram_tensor("encn", [tok, IN1], U16, kind="ExternalInput")
    w2t = nc.dram_tensor("w2t", [IN2, H], U16, kind="ExternalInput")
    h0t = nc.dram_tensor("h0t", [IN2, bc], U16, kind="ExternalInput")
    b2r = nc.dram_tensor("b2r", [1, H], U16, kind="ExternalInput")
    vbr = nc.dram_tensor("vbr", [128, 4], U16, kind="ExternalInput")
    onum = nc.dram_tensor("onum", [128, 3 * bc], F32, kind="ExternalOutput")

    Tanh = mybir.ActivationFunctionType.Tanh
    Exp = mybir.ActivationFunctionType.Exp
    DR = mybir.MatmulPerfMode.DoubleRow

    with tile.TileContext(nc) as tc, ExitStack() as ctx:
        consts = ctx.enter_context(tc.tile_pool(name="consts", bufs=1))
        qpool = ctx.enter_context(tc.tile_pool(name="qpool", bufs=2))
        upool = ctx.enter_context(tc.tile_pool(name="upool", bufs=2))
        enp = ctx.enter_context(tc.tile_pool(name="enp", bufs=2))
        epool = ctx.enter_context(tc.tile_pool(name="epool", bufs=2))
        opool = ctx.enter_context(tc.tile_pool(name="opool", bufs=1))
        zpool = ctx.enter_context(tc.tile_pool(name="zpool", bufs=3, space="PSUM"))
        lpool = ctx.enter_context(tc.tile_pool(name="lpool", bufs=2, space="PSUM"))

        # ---------------- prologue: constants ----------------
        w1a = consts.tile([128, 2, 4, 128], F8E4)
        nc.sync.dma_start(out=w1a.bitcast(U8), in_=w1p1[:, :])
        w1b = consts.tile([128, 2, 4, 128], F8E5)
        nc.sync.dma_start(out=w1b.bitcast(U8), in_=w1p2[:, :])
        vbt = consts.tile([128, 4], BF16)
        nc.sync.dma_start(out=vbt.bitcast(U16), in_=vbr[:, :])
        w2s = consts.tile([128, 4, H], BF16)
        for k in range(4):
            nc.sync.dma_start(out=w2s[:, k, :].bitcast(U16),
                              in_=w2t[k * 128:(k + 1) * 128, :])
        h0s = consts.tile([128, 4, bc], BF16)
        for k in range(4):
            nc.sync.dma_start(out=h0s[:, k, :].bitcast(U16),
                              in_=h0t[k * 128:(k + 1) * 128, :])
        b2s = consts.tile([1, H], BF16)
        nc.sync.dma_start(out=b2s.bitcast(U16), in_=b2r[:, :])
        ones_col = consts.tile([128, 1], BF16)
        nc.vector.memset(ones_col, 1.0)
        ones_row = consts.tile([1, bc], BF16)
        nc.vector.memset(ones_row, 1.0)

        # c = (W2 h0 + b2), H-major: [128 H-in-chunk, chunk, batch]
        # (borrows an lpool slot so PSUM stays within 8 banks)
        psum_c = lpool.tile([128, 512], F32, tag="lt")
        for j in range(4):
            cs = psum_c[:, j * bc:(j + 1) * bc]
            for k in range(4):
                nc.tensor.matmul(cs, w2s[:, k, j * 128:(j + 1) * 128],
                                 h0s[:, k, :], start=(k == 0), stop=False)
            nc.tensor.matmul(cs, b2s[0:1, j * 128:(j + 1) * 128],
                             ones_row, start=False, stop=True)
        c_sb = consts.tile([128, 4, bc], F32)
        nc.vector.tensor_copy(
            c_sb, psum_c[:, 0:4 * bc].rearrange("p (j b) -> p j b", j=4))

        osb = opool.tile([128, 3, bc], F32)
        nc.vector.memset(osb, 0.0)

        # ---------------- main pipeline ----------------
        for b in range(bc):
            # logits + finals psum for this batch (one full PSUM bank so the
            # two lpool slots land in different zero regions):
            #   cols 0..15  logits (token-major, tile s)
            #   cols 16,17  output numerator IN1-chunks
            #   col  18     softmax denominator S (partition 0)
            psum_lt = lpool.tile([128, 512], F32, tag="lt")
            en_sb = enp.tile([128, N // 128, IN1], BF16, tag="en")
            nc.gpsimd.dma_start(
                out=en_sb.bitcast(U16),
                in_=encn[b * N:(b + 1) * N, :].rearrange(
                    "(s p) c -> p s c", p=128))
            for hh in range(2):
                hb = 2 * b + hh
                t0 = hb * HB
                q1_sb = qpool.tile([128, 2, HB], F8E4, tag="q1")
                q2_sb = qpool.tile([128, 2, HB], F8E4, tag="q2")
                nc.sync.dma_start(
                    out=q1_sb.bitcast(U8),
                    in_=bass.AP(tensor=q1d, offset=t0,
                                ap=[[2 * tok, 128], [tok, 2], [1, HB]]))
                nc.sync.dma_start(
                    out=q2_sb.bitcast(U8),
                    in_=bass.AP(tensor=q2d, offset=t0,
                                ap=[[2 * tok, 128], [tok, 2], [1, HB]]))
                u_sb = upool.tile([128, 4, HB], BF16, tag="u")
                for j in range(4):
                    pz = zpool.tile([128, HB], F32, tag="z")
                    for v in range(HB // 512):   # psum-bank-sized outputs
                        pzv = pz[:, v * 512:(v + 1) * 512]
                        qs = slice(v * 512, (v + 1) * 512)
                        nc.tensor.matmul(pzv, w1a[:, :, j, :], q1_sb[:, :, qs],
                                         start=True, stop=False, perf_mode=DR)
                        nc.tensor.matmul(pzv, w1b[:, :, j, :], q1_sb[:, :, qs],
                                         start=False, stop=False, perf_mode=DR)
                        nc.tensor.matmul(pzv, w1a[:, :, j, :], q2_sb[:, :, qs],
                                         start=False, stop=True, perf_mode=DR)
                    nc.scalar.activation(u_sb[:, j, :], pz, Tanh,
                                         bias=c_sb[:, j, b:b + 1])
                for st in range(HB // 128):   # V-dot, 8 token tiles
                    s = hh * (HB // 128) + st
                    for j in range(4):
                        nc.tensor.matmul(
                            psum_lt[:, s:s + 1],
                            u_sb[:, j, st * 128:(st + 1) * 128],
                            vbt[:, j:j + 1],
                            start=(j == 0), stop=(j == 3))
            e_sb = epool.tile([128, N // 128], BF16, tag="e")
            nc.scalar.activation(e_sb, psum_lt[:, 0:N // 128], Exp)
            # finals: three SEQUENTIAL accumulation groups (interleaved
            # groups in one psum zero-region are illegal)
            for s in range(N // 128):
                nc.tensor.matmul(psum_lt[:, 16:17], en_sb[:, s, 0:128],
                                 e_sb[:, s:s + 1],
                                 start=(s == 0), stop=(s == N // 128 - 1))
            for s in range(N // 128):
                nc.tensor.matmul(psum_lt[:, 17:18], en_sb[:, s, 128:256],
                                 e_sb[:, s:s + 1],
                                 start=(s == 0), stop=(s == N // 128 - 1))
            for s in range(N // 128):
                nc.tensor.matmul(psum_lt[0:1, 18:19], ones_col,
                                 e_sb[:, s:s + 1],
                                 start=(s == 0), stop=(s == N // 128 - 1))
            nc.vector.tensor_copy(osb[:, 0:2, b], psum_lt[:, 16:18])
            nc.vector.tensor_copy(osb[0:1, 2, b:b + 1], psum_lt[0:1, 18:19])

        nc.sync.dma_start(out=onum[:, :],
                          in_=osb.rearrange("p c b -> p (c b)"))

    return nc


def _bits16(x):
    return np.ascontiguousarray(x.astype(ml_dtypes.bfloat16)).view(np.uint16)


def kernel(**inputs):
    global LAST_RUNNER, _CACHED_NC
    enc = np.asarray(inputs["enc_outputs"], dtype=np.float32)   # [B, N, IN1]
    h0 = np.asarray(inputs["h0"], dtype=np.float32)             # [B, IN2]
    W1 = np.asarray(inputs["W1"], dtype=np.float32)             # [H, IN1]
    W2 = np.asarray(inputs["W2"], dtype=np.float32)             # [H, IN2]
    b2 = np.asarray(inputs["b2"], dtype=np.float32)             # [H]
    V = np.asarray(inputs["V"], dtype=np.float32)               # [H, 1]

    E4, E5 = ml_dtypes.float8_e4m3, ml_dtypes.float8_e5m2

    # W1 fp8 + residual, DoubleRow layout [p, plane, chunk, h]:
    # value = W1[chunk*128 + h, plane*128 + p]
    p1 = W1.astype(E4)
    p2 = (W1 - p1.astype(np.float32)).astype(E5)
    def w1_dr(q):
        # [H, IN1] -> [IN1-part 128, plane 2, chunk 4, h 128]
        a = q.reshape(4, 128, 2, 128).transpose(3, 2, 0, 1)
        return np.ascontiguousarray(a).view(np.uint8).reshape(128, 2 * 4 * 128)
    w1p1 = w1_dr(p1)
    w1p2 = w1_dr(p2)

    w2t = _bits16(W2.T)                                         # [IN2, H]
    b2r = _bits16(b2.reshape(1, H))
    vbr = _bits16(V.reshape(4, 128).T)                          # [128, 4]

    in_maps = []
    for c in range(NCORES):
        enc_c = enc[c * BC:(c + 1) * BC].reshape(TOK, IN1)      # [tok, 256]
        q1 = enc_c.astype(E4)
        q2 = (enc_c - q1.astype(np.float32)).astype(E4)
        def enc_dr(q):
            # [tok, IN1] -> [p 128, plane 2, tok]
            a = q.view(np.uint8).T.reshape(2, 128, TOK).transpose(1, 0, 2)
            return np.ascontiguousarray(a).reshape(128, 2 * TOK)
        h0t = _bits16(h0[c * BC:(c + 1) * BC].T)                # [IN2, 16]
        in_maps.append({
            "q1d": enc_dr(q1), "q2d": enc_dr(q2),
            "w1p1": w1p1, "w1p2": w1p2,
            "encn": _bits16(enc_c), "w2t": w2t, "h0t": h0t,
            "b2r": b2r, "vbr": vbr,
        })

    if _CACHED_NC is None:
        _CACHED_NC = build_nc()
    nc = _CACHED_NC

    runner = Runner(nc, in_maps)
    LAST_RUNNER = runner
    results = runner.outputs(runner.run())
    outs = []
    for c in range(NCORES):
        onum = results[c]["onum"].reshape(128, 3, BC)
        num = onum[:, 0:2, :].transpose(2, 1, 0).reshape(BC, IN1)
        s = onum[0, 2, :]                                       # [bc]
        outs.append(num / s[:, None])
    return np.concatenate(outs, axis=0).astype(np.float32)


# revision 39
# speedup vs baseline: 1.4428x; 1.0406x over previous
"""Bass/Trainium2 kernel for nn_Attention (Bahdanau-style attention).

  w1e   = enc @ W1.T                      [B, N, H]
  w2h   = h0 @ W2.T + b2                  [B, H]
  u     = tanh(w1e + w2h[:, None, :])     [B, N, H]
  logits= u @ V                           [B, N, 1]
  att   = softmax(logits, axis=1)
  out   = att^T @ enc                     [B, IN1]

Sharding: pure data-parallel over batch B=128 across 8 cores (16 batches
each); W1/W2/V replicated. No collectives.

Per-core dataflow (H-major main matmul, fp8 residual quantization):
  - main matmul runs H-on-partitions: psum z[128 Hchunk, 1024 tok] per
    (chunk, half-batch).  enc and W1 are quantized to fp8 as q1=e4m3(x),
    plus raw residuals q2=e4m3(enc-q1), p2=e5m2(W1-p1); z = q1@p1 +
    q2@p1 + q1@p2 (the q2@p2 term is negligible).  Each term is ONE
    DoubleRow matmul contracting K=256 via 2 fp8 planes (measured w1e
    rms err 1.6e-3, better than bf16's 2.4e-3).
  - c = W2 h0 + b2 is computed H-major on the PE at startup
    ([128 H, 16 batches] psum) and folded into tanh as the per-partition
    activation BIAS - no PE/DVE cost in the main loop.
  - tanh on ScalarE per (chunk, half-batch): [128, 1024] psum -> sbuf
    u bf16.
  - V-dot uses u as the STATIONARY operand ([128 H, 128 tok] tiles) and
    V[chunk] as the 1-column MOVING operand: out [128 tok, 1] psum
    columns accumulate K=H over 4 chunk-matmuls.  Output lands
    token-major ([128, 16] logits per batch) so exp is a tiny ScalarE op.
  - finals: stationary = enc natural tile (bf16) [128 tok, 128 IN1],
    moving = e column [128,1] -> psum [128, 1] per IN1-chunk; plus an
    all-ones stationary for the softmax denominator S.  All finals
    matmuls have free-size 1.
  - numerators + S are staged to SBUF and shipped out once; the final
    divide by S happens on host during unsharding.
"""

import os
import sys

for _p in ("/opt/trn_rl_repo",):
    if _p not in sys.path and os.path.isdir(_p):
        sys.path.insert(0, _p)

from contextlib import ExitStack

import ml_dtypes
import numpy as np

import concourse.bass as bass
from concourse import bacc, mybir, tile

B, N, IN1, IN2, H = 128, 2048, 256, 512, 512
NCORES = 8
BC = B // NCORES            # 16 batches per core
TOK = BC * N                # 32768 tokens per core
HB = 1024                   # tokens per half-batch block
NHB = TOK // HB             # 32 half-batch blocks per core

F32 = mybir.dt.float32
BF16 = mybir.dt.bfloat16
F8E4 = mybir.dt.float8e4
F8E5 = mybir.dt.float8e5
U16 = mybir.dt.uint16
U8 = mybir.dt.uint8

LAST_RUNNER = None

_CACHED_NC = None


class Runner:
    """Compile-once SPMD runner (replicates run_bass_via_pjrt's multi-core
    path) that keeps the jitted callable + device-resident inputs so
    repeated executions can be wall-clocked without compile/transfer."""

    def __init__(self, nc, in_maps):
        import jax
        from jax.experimental.shard_map import shard_map
        from jax.sharding import Mesh, NamedSharding, PartitionSpec

        from concourse import bass2jax, mybir as _mybir

        bass2jax.install_neuronx_cc_hook()
        self.jax = jax

        if not nc.is_finalized():
            nc.finalize()

        partition_name = (nc.partition_id_tensor.name
                          if nc.partition_id_tensor else None)
        in_names, out_names, out_avals, zero_outs = [], [], [], []
        for alloc in nc.m.functions[0].allocations:
            if not isinstance(alloc, _mybir.MemoryLocationSet):
                continue
            name = alloc.memorylocations[0].name
            if alloc.kind == "ExternalInput":
                if name != partition_name:
                    in_names.append(name)
            elif alloc.kind == "ExternalOutput":
                shape = tuple(alloc.tensor_shape)
                dtype = _mybir.dt.np(alloc.dtype)
                out_names.append(name)
                out_avals.append(jax.core.ShapedArray(shape, dtype))
                zero_outs.append(np.zeros(shape, dtype))
        n_params = len(in_names)
        all_in_names = list(in_names) + list(out_names)
        if partition_name is not None:
            all_in_names.append(partition_name)
        self.out_names = out_names

        def _body(*args):
            operands = list(args)
            if partition_name is not None:
                operands.append(bass2jax.partition_id_tensor())
            outs = bass2jax._bass_exec_p.bind(
                *operands,
                out_avals=tuple(out_avals),
                in_names=tuple(all_in_names),
                out_names=tuple(out_names),
                lowering_input_output_aliases=(),
                sim_require_finite=True,
                sim_require_nnan=True,
                nc=nc,
            )
            return tuple(outs)

        n_cores = len(in_maps)
        devices = jax.devices()[:n_cores]
        mesh = Mesh(np.asarray(devices), ("core",))
        spec = PartitionSpec("core")
        self.n_cores = n_cores
        self.out_avals = out_avals
        self.sharded = jax.jit(
            shard_map(_body, mesh=mesh,
                      in_specs=(spec,) * (n_params + len(out_names)),
                      out_specs=(spec,) * len(out_names),
                      check_rep=False),
            keep_unused=True,
        )

        self._n_params = n_params
        sharding = NamedSharding(mesh, spec)
        self.dev_in = [
            jax.device_put(
                np.concatenate([np.asarray(in_maps[c][nm])
                                for c in range(n_cores)], axis=0), sharding)
            for nm in in_names
        ]
        self.dev_zeros = [
            jax.device_put(
                np.zeros((n_cores * z.shape[0], *z.shape[1:]), z.dtype), sharding)
            for z in zero_outs
        ]

    def run(self):
        out = self.sharded(*self.dev_in, *self.dev_zeros)
        self.jax.block_until_ready(out)
        return out

    def run_chain(self, k):
        # k async dispatches of the same executable; PJRT serializes them
        # on the device stream.
        out = None
        for _ in range(k):
            out = self.sharded(*self.dev_in, *self.dev_zeros)
        self.jax.block_until_ready(out)
        return out

    def outputs(self, out_arrs):
        return [
            {nm: np.asarray(out_arrs[i]).reshape(
                self.n_cores, *self.out_avals[i].shape)[c]
             for i, nm in enumerate(self.out_names)}
            for c in range(self.n_cores)
        ]


def build_nc(bc=BC):
    tok = bc * N
    nhb = tok // HB
    nc = bacc.Bacc(None, target_bir_lowering=False)

    # NOTE: native bf16/fp8 ExternalInputs are mangled by the axon/PJRT
    # transfer path; ship raw bits as uint16/uint8 and bitcast on-chip.
    q1d = nc.dram_tensor("q1d", [128, 2 * tok], U8, kind="ExternalInput")
    q2d = nc.dram_tensor("q2d", [128, 2 * tok], U8, kind="ExternalInput")
    w1p1 = nc.dram_tensor("w1p1", [128, 2 * 4 * 128], U8, kind="ExternalInput")
    w1p2 = nc.dram_tensor("w1p2", [128, 2 * 4 * 128], U8, kind="ExternalInput")
    encn = nc.dram_tensor("encn", [tok, IN1], U16, kind="ExternalInput")
    w2t = nc.dram_tensor("w2t", [IN2, H], U16, kind="ExternalInput")
    h0t = nc.dram_tensor("h0t", [IN2, bc], U16, kind="ExternalInput")
    b2r = nc.dram_tensor("b2r", [1, H], U16, kind="ExternalInput")
    vbr = nc.dram_tensor("vbr", [128, 4], U16, kind="ExternalInput")
    eyed = nc.dram_tensor("eyed", [bc, bc], U8, kind="ExternalInput")
    onum = nc.dram_tensor("onum", [128, 3 * bc], F32, kind="ExternalOutput")

    Tanh = mybir.ActivationFunctionType.Tanh
    Exp = mybir.ActivationFunctionType.Exp
    DR = mybir.MatmulPerfMode.DoubleRow

    with tile.TileContext(nc) as tc, ExitStack() as ctx:
        consts = ctx.enter_context(tc.tile_pool(name="consts", bufs=1))
        qpool = ctx.enter_context(tc.tile_pool(name="qpool", bufs=3))
        upool = ctx.enter_context(tc.tile_pool(name="upool", bufs=2))
        enp = ctx.enter_context(tc.tile_pool(name="enp", bufs=3))
        epool = ctx.enter_context(tc.tile_pool(name="epool", bufs=2))
        opool = ctx.enter_context(tc.tile_pool(name="opool", bufs=1))
        zpool = ctx.enter_context(tc.tile_pool(name="zpool", bufs=3, space="PSUM"))
        lpool = ctx.enter_context(tc.tile_pool(name="lpool", bufs=1, space="PSUM"))

        # ---------------- prologue: constants ----------------
        # w1/q DMAs go on SP (first in its queue -> mains start early);
        # the c-chain constants go on the ACT-issued HWDGE path so they
        # don't queue behind the big q streams.
        w1a = consts.tile([128, 2, 4, 128], F8E4)
        nc.sync.dma_start(out=w1a.bitcast(U8), in_=w1p1[:, :])
        w1b = consts.tile([128, 2, 4, 128], F8E5)
        nc.sync.dma_start(out=w1b.bitcast(U8), in_=w1p2[:, :])
        vbt = consts.tile([128, 4], BF16)
        nc.scalar.dma_start(out=vbt.bitcast(U16), in_=vbr[:, :])
        w2s = consts.tile([128, 4, H], BF16)
        nc.scalar.dma_start(
            out=w2s.bitcast(U16),
            in_=w2t[:, :].rearrange("(k p) h -> p k h", p=128))
        h0s = consts.tile([128, 4, bc], BF16)
        nc.scalar.dma_start(
            out=h0s.bitcast(U16),
            in_=h0t[:, :].rearrange("(k p) b -> p k b", p=128))
        b2s = consts.tile([1, H], BF16)
        nc.scalar.dma_start(out=b2s.bitcast(U16), in_=b2r[:, :])
        ones_col = consts.tile([128, 1], BF16)
        nc.vector.memset(ones_col, 1.0)
        ones_row = consts.tile([1, bc], BF16)
        nc.vector.memset(ones_row, 1.0)

        # c = (W2 h0 + b2), batch-major: [16 batches (partitions), 512 H].
        # It is folded into the mains PSUM accumulation by a K=1 DoubleRow
        # matmul per 512-token slice (stationary = fp8 c planes, moving =
        # ones), so tanh needs no per-chunk bias and can span chunks.
        # (borrows an lpool slot so PSUM stays within 8 banks)
        psum_c = lpool.tile([128, 512], F32, tag="lt")
        cs = psum_c[0:bc, :]
        for k in range(4):
            nc.tensor.matmul(cs, h0s[:, k, :], w2s[:, k, :],
                             start=(k == 0), stop=False)
        nc.tensor.matmul(cs, ones_row, b2s, start=False, stop=True)
        # fp8 split c = c1 + c2, both e4m3 (one dtype per AP; the residual
        # |c-c1| <~ 0.07 sits near e4m3 min-normal, err ~4e-3); bounce via
        # DRAM to partition-0-major so every c-fold stationary reads
        # partition 0.
        ccomb = consts.tile([bc, 2, H], F8E4)
        ctmp = consts.tile([bc, H], F32)
        nc.vector.tensor_copy(ccomb[:, 0, :], cs)
        nc.vector.tensor_tensor(out=ctmp, in0=cs, in1=ccomb[:, 0, :],
                                op=mybir.AluOpType.subtract)
        nc.vector.tensor_copy(ccomb[:, 1, :], ctmp)
        # identity selector: the c-fold matmul contracts K=16 batches and
        # a stride-0 AP over column b of eye16 picks batch b's c planes
        eye16 = consts.tile([bc, bc], F8E4)
        nc.gpsimd.dma_start(out=eye16.bitcast(U8), in_=eyed[:, :])

        osb = opool.tile([128, 3, bc], F32)
        nc.vector.memset(osb, 0.0)

        # ---------------- main pipeline ----------------
        # The batch is cut into 16 (H-chunk, 512-token) slices; each slice
        # accumulates 3 main DoubleRow matmuls + 1 c-fold DoubleRow matmul
        # in one PSUM bank.  Since c is folded on the PE, a tanh
        # instruction can span chunks: slices are tanh'd 3 at a time from
        # 3-bank [128, 1536] psum tiles (2 bufs = 6 banks + 2 logit banks).
        # The V-dot / exp / finals for batch b are ISSUED one batch late,
        # so the in-order PE queue never parks on a V-dot matmul waiting
        # for tanh while the next mains could run.
        us, lts, ens = {}, {}, {}

        def mains(pzv, q1s, q2s, j, b):
            nc.tensor.matmul(pzv, w1a[:, :, j, :], q1s,
                             start=True, stop=False, perf_mode=DR)
            nc.tensor.matmul(pzv, w1b[:, :, j, :], q1s,
                             start=False, stop=False, perf_mode=DR)
            nc.tensor.matmul(pzv, w1a[:, :, j, :], q2s,
                             start=False, stop=False, perf_mode=DR)
            # c-fold: K=16 contraction vs eye column b selects c1_b + c2_b
            sel = bass.AP(tensor=eye16.tensor, offset=eye16.offset + b,
                          ap=[list(eye16.ap[0]), [0, 2], [0, 512]])
            nc.tensor.matmul(pzv, ccomb[:, :, j * 128:(j + 1) * 128],
                             sel, start=False, stop=True, perf_mode=DR)

        ebs = {}

        def issue_vdot(b):
            # logits + finals psum for batch b (one full PSUM bank so the
            # two lpool slots land in different zero regions):
            #   cols 0..15  logits (token-major, tile s)
            #   cols 16,17  output numerator IN1-chunks
            #   col  18     softmax denominator S (partition 0)
            psum_lt = lpool.tile([128, 512], F32, tag="lt", name=f"lt{b}")
            lts[b] = psum_lt
            u_sb = us.pop(b)
            for s in range(N // 128):     # V-dot, 16 token tiles
                for j in range(4):
                    g, o = j * 4 + s // 4, (s % 4) * 128
                    nc.tensor.matmul(
                        psum_lt[:, s:s + 1],
                        u_sb[:, g, o:o + 128],
                        vbt[:, j:j + 1],
                        start=(j == 0), stop=(j == 3))
            e_sb = epool.tile([128, N // 128], BF16, tag="e")
            nc.scalar.activation(e_sb, psum_lt[:, 0:N // 128], Exp)
            ebs[b] = e_sb

        def issue_finals(b):
            psum_lt, e_sb, en_sb = lts.pop(b), ebs.pop(b), ens.pop(b)
            # finals: three SEQUENTIAL accumulation groups (interleaved
            # groups in one psum zero-region are illegal)
            for s in range(N // 128):
                nc.tensor.matmul(psum_lt[:, 16:17], en_sb[:, s, 0:128],
                                 e_sb[:, s:s + 1],
                                 start=(s == 0), stop=(s == N // 128 - 1))
            for s in range(N // 128):
                nc.tensor.matmul(psum_lt[:, 17:18], en_sb[:, s, 128:256],
                                 e_sb[:, s:s + 1],
                                 start=(s == 0), stop=(s == N // 128 - 1))
            for s in range(N // 128):
                nc.tensor.matmul(psum_lt[0:1, 18:19], ones_col,
                                 e_sb[:, s:s + 1],
                                 start=(s == 0), stop=(s == N // 128 - 1))
            nc.vector.tensor_copy(osb[:, 0:2, b], psum_lt[:, 16:18])
            nc.vector.tensor_copy(osb[0:1, 2, b:b + 1], psum_lt[0:1, 18:19])

        for b in range(bc):
            t0 = b * N
            q1_sb = qpool.tile([128, 2, N], F8E4, tag="q1")
            q2_sb = qpool.tile([128, 2, N], F8E4, tag="q2")
            nc.sync.dma_start(
                out=q1_sb.bitcast(U8),
                in_=bass.AP(tensor=q1d, offset=t0,
                            ap=[[2 * tok, 128], [tok, 2], [1, N]]))
            nc.sync.dma_start(
                out=q2_sb.bitcast(U8),
                in_=bass.AP(tensor=q2d, offset=t0,
                            ap=[[2 * tok, 128], [tok, 2], [1, N]]))
            en_sb = enp.tile([128, N // 128, IN1], BF16, tag="en")
            nc.gpsimd.dma_start(
                out=en_sb.bitcast(U16),
                in_=encn[t0:t0 + N, :].rearrange("(s p) c -> p s c", p=128))
            ens[b] = en_sb
            u_sb = upool.tile([128, 16, 512], BF16, tag="u")
            us[b] = u_sb
            # 16 slices g = (chunk j = g//4, token quarter v = g%4),
            # tanh'd 3 slices at a time (last instr covers just 1)
            for g0 in range(0, 16, 3):
                k = min(3, 16 - g0)
                if k == 3:
                    pz = zpool.tile([128, 3 * 512], F32, tag="zb", bufs=2,
                                    name=f"zb{b}_{g0}")
                else:
                    pz = zpool.tile([128, 512], F32, tag="z1", bufs=1,
                                    name=f"z1{b}_{g0}")
                for i in range(k):
                    g = g0 + i
                    j, v = g // 4, g % 4
                    mains(pz[:, i * 512:(i + 1) * 512],
                          q1_sb[:, :, v * 512:(v + 1) * 512],
                          q2_sb[:, :, v * 512:(v + 1) * 512], j, b)
                nc.scalar.activation(
                    u_sb[:, g0:g0 + k, :].rearrange("p g t -> p (g t)"),
                    pz[:, 0:k * 512], Tanh)
                # interleave last batch's V-dot/exp after block 0 and its
                # finals after block 1, so the in-order PE/ACT queues
                # never park on them
                if b > 0 and g0 == 0:
                    issue_vdot(b - 1)
                if b > 0 and g0 == 3:
                    issue_finals(b - 1)
        issue_vdot(bc - 1)
        issue_finals(bc - 1)

        nc.sync.dma_start(out=onum[:, :],
                          in_=osb.rearrange("p c b -> p (c b)"))

    return nc


def _bits16(x):
    return np.ascontiguousarray(x.astype(ml_dtypes.bfloat16)).view(np.uint16)


def kernel(**inputs):
    global LAST_RUNNER, _CACHED_NC
    enc = np.asarray(inputs["enc_outputs"], dtype=np.float32)   # [B, N, IN1]
    h0 = np.asarray(inputs["h0"], dtype=np.float32)             # [B, IN2]
    W1 = np.asarray(inputs["W1"], dtype=np.float32)             # [H, IN1]
    W2 = np.asarray(inputs["W2"], dtype=np.float32)             # [H, IN2]
    b2 = np.asarray(inputs["b2"], dtype=np.float32)             # [H]
    V = np.asarray(inputs["V"], dtype=np.float32)               # [H, 1]

    E4, E5 = ml_dtypes.float8_e4m3, ml_dtypes.float8_e5m2

    # W1 fp8 + residual, DoubleRow layout [p, plane, chunk, h]:
    # value = W1[chunk*128 + h, plane*128 + p]
    p1 = W1.astype(E4)
    p2 = (W1 - p1.astype(np.float32)).astype(E5)
    def w1_dr(q):
        # [H, IN1] -> [IN1-part 128, plane 2, chunk 4, h 128]
        a = q.reshape(4, 128, 2, 128).transpose(3, 2, 0, 1)
        return np.ascontiguousarray(a).view(np.uint8).reshape(128, 2 * 4 * 128)
    w1p1 = w1_dr(p1)
    w1p2 = w1_dr(p2)

    w2t = _bits16(W2.T)                                         # [IN2, H]
    b2r = _bits16(b2.reshape(1, H))
    vbr = _bits16(V.reshape(4, 128).T)                          # [128, 4]
    eyed = np.ascontiguousarray(np.eye(BC, dtype=E4)).view(np.uint8)

    in_maps = []
    for c in range(NCORES):
        enc_c = enc[c * BC:(c + 1) * BC].reshape(TOK, IN1)      # [tok, 256]
        q1 = enc_c.astype(E4)
        q2 = (enc_c - q1.astype(np.float32)).astype(E4)
        def enc_dr(q):
            # [tok, IN1] -> [p 128, plane 2, tok]
            a = q.view(np.uint8).T.reshape(2, 128, TOK).transpose(1, 0, 2)
            return np.ascontiguousarray(a).reshape(128, 2 * TOK)
        h0t = _bits16(h0[c * BC:(c + 1) * BC].T)                # [IN2, 16]
        in_maps.append({
            "q1d": enc_dr(q1), "q2d": enc_dr(q2),
            "w1p1": w1p1, "w1p2": w1p2,
            "encn": _bits16(enc_c), "w2t": w2t, "h0t": h0t,
            "b2r": b2r, "vbr": vbr, "eyed": eyed,
        })

    if _CACHED_NC is None:
        _CACHED_NC = build_nc()
    nc = _CACHED_NC

    runner = Runner(nc, in_maps)
    LAST_RUNNER = runner
    results = runner.outputs(runner.run())
    outs = []
    for c in range(NCORES):
        onum = results[c]["onum"].reshape(128, 3, BC)
        num = onum[:, 0:2, :].transpose(2, 1, 0).reshape(BC, IN1)
        s = onum[0, 2, :]                                       # [bc]
        outs.append(num / s[:, None])
    return np.concatenate(outs, axis=0).astype(np.float32)


# revision 40
# speedup vs baseline: 1.4478x; 1.0035x over previous
"""Bass/Trainium2 kernel for nn_Attention (Bahdanau-style attention).

  w1e   = enc @ W1.T                      [B, N, H]
  w2h   = h0 @ W2.T + b2                  [B, H]
  u     = tanh(w1e + w2h[:, None, :])     [B, N, H]
  logits= u @ V                           [B, N, 1]
  att   = softmax(logits, axis=1)
  out   = att^T @ enc                     [B, IN1]

Sharding: pure data-parallel over batch B=128 across 8 cores (16 batches
each); W1/W2/V replicated. No collectives.

Per-core dataflow (H-major main matmul, fp8 residual quantization):
  - main matmul runs H-on-partitions: psum z[128 Hchunk, 1024 tok] per
    (chunk, half-batch).  enc and W1 are quantized to fp8 as q1=e4m3(x),
    plus raw residuals q2=e4m3(enc-q1), p2=e5m2(W1-p1); z = q1@p1 +
    q2@p1 + q1@p2 (the q2@p2 term is negligible).  Each term is ONE
    DoubleRow matmul contracting K=256 via 2 fp8 planes (measured w1e
    rms err 1.6e-3, better than bf16's 2.4e-3).
  - c = W2 h0 + b2 is computed H-major on the PE at startup
    ([128 H, 16 batches] psum) and folded into tanh as the per-partition
    activation BIAS - no PE/DVE cost in the main loop.
  - tanh on ScalarE per (chunk, half-batch): [128, 1024] psum -> sbuf
    u bf16.
  - V-dot uses u as the STATIONARY operand ([128 H, 128 tok] tiles) and
    V[chunk] as the 1-column MOVING operand: out [128 tok, 1] psum
    columns accumulate K=H over 4 chunk-matmuls.  Output lands
    token-major ([128, 16] logits per batch) so exp is a tiny ScalarE op.
  - finals: stationary = enc natural tile (bf16) [128 tok, 128 IN1],
    moving = e column [128,1] -> psum [128, 1] per IN1-chunk; plus an
    all-ones stationary for the softmax denominator S.  All finals
    matmuls have free-size 1.
  - numerators + S are staged to SBUF and shipped out once; the final
    divide by S happens on host during unsharding.
"""

import os
import sys

for _p in ("/opt/trn_rl_repo",):
    if _p not in sys.path and os.path.isdir(_p):
        sys.path.insert(0, _p)

from contextlib import ExitStack

import ml_dtypes
import numpy as np

import concourse.bass as bass
from concourse import bacc, mybir, tile

B, N, IN1, IN2, H = 128, 2048, 256, 512, 512
NCORES = 8
BC = B // NCORES            # 16 batches per core
TOK = BC * N                # 32768 tokens per core
HB = 1024                   # tokens per half-batch block
NHB = TOK // HB             # 32 half-batch blocks per core

F32 = mybir.dt.float32
BF16 = mybir.dt.bfloat16
F8E4 = mybir.dt.float8e4
F8E5 = mybir.dt.float8e5
U16 = mybir.dt.uint16
U8 = mybir.dt.uint8

LAST_RUNNER = None

_CACHED_NC = None


class Runner:
    """Compile-once SPMD runner (replicates run_bass_via_pjrt's multi-core
    path) that keeps the jitted callable + device-resident inputs so
    repeated executions can be wall-clocked without compile/transfer."""

    def __init__(self, nc, in_maps):
        import jax
        from jax.experimental.shard_map import shard_map
        from jax.sharding import Mesh, NamedSharding, PartitionSpec

        from concourse import bass2jax, mybir as _mybir

        bass2jax.install_neuronx_cc_hook()
        self.jax = jax

        if not nc.is_finalized():
            nc.finalize()

        partition_name = (nc.partition_id_tensor.name
                          if nc.partition_id_tensor else None)
        in_names, out_names, out_avals, zero_outs = [], [], [], []
        for alloc in nc.m.functions[0].allocations:
            if not isinstance(alloc, _mybir.MemoryLocationSet):
                continue
            name = alloc.memorylocations[0].name
            if alloc.kind == "ExternalInput":
                if name != partition_name:
                    in_names.append(name)
            elif alloc.kind == "ExternalOutput":
                shape = tuple(alloc.tensor_shape)
                dtype = _mybir.dt.np(alloc.dtype)
                out_names.append(name)
                out_avals.append(jax.core.ShapedArray(shape, dtype))
                zero_outs.append(np.zeros(shape, dtype))
        n_params = len(in_names)
        all_in_names = list(in_names) + list(out_names)
        if partition_name is not None:
            all_in_names.append(partition_name)
        self.out_names = out_names

        def _body(*args):
            operands = list(args)
            if partition_name is not None:
                operands.append(bass2jax.partition_id_tensor())
            outs = bass2jax._bass_exec_p.bind(
                *operands,
                out_avals=tuple(out_avals),
                in_names=tuple(all_in_names),
                out_names=tuple(out_names),
                lowering_input_output_aliases=(),
                sim_require_finite=True,
                sim_require_nnan=True,
                nc=nc,
            )
            return tuple(outs)

        n_cores = len(in_maps)
        devices = jax.devices()[:n_cores]
        mesh = Mesh(np.asarray(devices), ("core",))
        spec = PartitionSpec("core")
        self.n_cores = n_cores
        self.out_avals = out_avals
        self.sharded = jax.jit(
            shard_map(_body, mesh=mesh,
                      in_specs=(spec,) * (n_params + len(out_names)),
                      out_specs=(spec,) * len(out_names),
                      check_rep=False),
            keep_unused=True,
        )

        self._n_params = n_params
        sharding = NamedSharding(mesh, spec)
        self.dev_in = [
            jax.device_put(
                np.concatenate([np.asarray(in_maps[c][nm])
                                for c in range(n_cores)], axis=0), sharding)
            for nm in in_names
        ]
        self.dev_zeros = [
            jax.device_put(
                np.zeros((n_cores * z.shape[0], *z.shape[1:]), z.dtype), sharding)
            for z in zero_outs
        ]

    def run(self):
        out = self.sharded(*self.dev_in, *self.dev_zeros)
        self.jax.block_until_ready(out)
        return out

    def run_chain(self, k):
        # k async dispatches of the same executable; PJRT serializes them
        # on the device stream.
        out = None
        for _ in range(k):
            out = self.sharded(*self.dev_in, *self.dev_zeros)
        self.jax.block_until_ready(out)
        return out

    def outputs(self, out_arrs):
        return [
            {nm: np.asarray(out_arrs[i]).reshape(
                self.n_cores, *self.out_avals[i].shape)[c]
             for i, nm in enumerate(self.out_names)}
            for c in range(self.n_cores)
        ]


def build_nc(bc=BC):
    tok = bc * N
    nhb = tok // HB
    nc = bacc.Bacc(None, target_bir_lowering=False)

    # NOTE: native bf16/fp8 ExternalInputs are mangled by the axon/PJRT
    # transfer path; ship raw bits as uint16/uint8 and bitcast on-chip.
    q1d = nc.dram_tensor("q1d", [128, 2 * tok], U8, kind="ExternalInput")
    q2d = nc.dram_tensor("q2d", [128, 2 * tok], U8, kind="ExternalInput")
    w1p1 = nc.dram_tensor("w1p1", [128, 2 * 4 * 128], U8, kind="ExternalInput")
    w1p2 = nc.dram_tensor("w1p2", [128, 2 * 4 * 128], U8, kind="ExternalInput")
    encn = nc.dram_tensor("encn", [tok, IN1], U16, kind="ExternalInput")
    w2t = nc.dram_tensor("w2t", [IN2, H], U16, kind="ExternalInput")
    h0t = nc.dram_tensor("h0t", [IN2, bc], U16, kind="ExternalInput")
    b2r = nc.dram_tensor("b2r", [1, H], U16, kind="ExternalInput")
    vbr = nc.dram_tensor("vbr", [128, 4], U16, kind="ExternalInput")
    eyed = nc.dram_tensor("eyed", [bc, bc], U8, kind="ExternalInput")
    onum = nc.dram_tensor("onum", [128, 3 * bc], F32, kind="ExternalOutput")

    Tanh = mybir.ActivationFunctionType.Tanh
    Exp = mybir.ActivationFunctionType.Exp
    DR = mybir.MatmulPerfMode.DoubleRow

    with tile.TileContext(nc) as tc, ExitStack() as ctx:
        consts = ctx.enter_context(tc.tile_pool(name="consts", bufs=1))
        qpool = ctx.enter_context(tc.tile_pool(name="qpool", bufs=3))
        upool = ctx.enter_context(tc.tile_pool(name="upool", bufs=2))
        enp = ctx.enter_context(tc.tile_pool(name="enp", bufs=3))
        epool = ctx.enter_context(tc.tile_pool(name="epool", bufs=2))
        opool = ctx.enter_context(tc.tile_pool(name="opool", bufs=1))
        zpool = ctx.enter_context(tc.tile_pool(name="zpool", bufs=3, space="PSUM"))
        lpool = ctx.enter_context(tc.tile_pool(name="lpool", bufs=1, space="PSUM"))

        # ---------------- prologue: constants ----------------
        # w1/q DMAs go on SP (first in its queue -> mains start early);
        # the c-chain constants go on the ACT-issued HWDGE path so they
        # don't queue behind the big q streams.
        w1a = consts.tile([128, 2, 4, 128], F8E4)
        nc.sync.dma_start(out=w1a.bitcast(U8), in_=w1p1[:, :])
        w1b = consts.tile([128, 2, 4, 128], F8E5)
        nc.sync.dma_start(out=w1b.bitcast(U8), in_=w1p2[:, :])
        w2s = consts.tile([128, 4, H], BF16)
        nc.scalar.dma_start(
            out=w2s.bitcast(U16),
            in_=w2t[:, :].rearrange("(k p) h -> p k h", p=128))
        h0s = consts.tile([128, 4, bc], BF16)
        nc.scalar.dma_start(
            out=h0s.bitcast(U16),
            in_=h0t[:, :].rearrange("(k p) b -> p k b", p=128))
        b2s = consts.tile([1, H], BF16)
        nc.scalar.dma_start(out=b2s.bitcast(U16), in_=b2r[:, :])
        vbt = consts.tile([128, 4], BF16)
        nc.scalar.dma_start(out=vbt.bitcast(U16), in_=vbr[:, :])
        ones_col = consts.tile([128, 1], BF16)
        nc.vector.memset(ones_col, 1.0)
        ones_row = consts.tile([1, bc], BF16)
        nc.vector.memset(ones_row, 1.0)

        # c = (W2 h0 + b2), batch-major: [16 batches (partitions), 512 H].
        # It is folded into the mains PSUM accumulation by a K=1 DoubleRow
        # matmul per 512-token slice (stationary = fp8 c planes, moving =
        # ones), so tanh needs no per-chunk bias and can span chunks.
        # (borrows an lpool slot so PSUM stays within 8 banks)
        psum_c = lpool.tile([128, 512], F32, tag="lt")
        cs = psum_c[0:bc, :]
        for k in range(4):
            nc.tensor.matmul(cs, h0s[:, k, :], w2s[:, k, :],
                             start=(k == 0), stop=False)
        nc.tensor.matmul(cs, ones_row, b2s, start=False, stop=True)
        # fp8 split c = c1 + c2, both e4m3 (one dtype per AP; the residual
        # |c-c1| <~ 0.07 sits near e4m3 min-normal, err ~4e-3); bounce via
        # DRAM to partition-0-major so every c-fold stationary reads
        # partition 0.
        ccomb = consts.tile([bc, 2, H], F8E4)
        ctmp = consts.tile([bc, H], F32)
        nc.vector.tensor_copy(ccomb[:, 0, :], cs)
        nc.vector.tensor_tensor(out=ctmp, in0=cs, in1=ccomb[:, 0, :],
                                op=mybir.AluOpType.subtract)
        nc.vector.tensor_copy(ccomb[:, 1, :], ctmp)
        # identity selector: the c-fold matmul contracts K=16 batches and
        # a stride-0 AP over column b of eye16 picks batch b's c planes
        eye16 = consts.tile([bc, bc], F8E4)
        nc.gpsimd.dma_start(out=eye16.bitcast(U8), in_=eyed[:, :])

        osb = opool.tile([128, 3, bc], F32)
        nc.vector.memset(osb, 0.0)

        # ---------------- main pipeline ----------------
        # The batch is cut into 16 (H-chunk, 512-token) slices; each slice
        # accumulates 3 main DoubleRow matmuls + 1 c-fold DoubleRow matmul
        # in one PSUM bank.  Since c is folded on the PE, a tanh
        # instruction can span chunks: slices are tanh'd 3 at a time from
        # 3-bank [128, 1536] psum tiles (2 bufs = 6 banks + 2 logit banks).
        # The V-dot / exp / finals for batch b are ISSUED one batch late,
        # so the in-order PE queue never parks on a V-dot matmul waiting
        # for tanh while the next mains could run.
        us, lts, ens = {}, {}, {}

        def mains(pzv, q1s, q2s, j, b):
            nc.tensor.matmul(pzv, w1a[:, :, j, :], q1s,
                             start=True, stop=False, perf_mode=DR)
            nc.tensor.matmul(pzv, w1b[:, :, j, :], q1s,
                             start=False, stop=False, perf_mode=DR)
            nc.tensor.matmul(pzv, w1a[:, :, j, :], q2s,
                             start=False, stop=False, perf_mode=DR)
            # c-fold: K=16 contraction vs eye column b selects c1_b + c2_b
            sel = bass.AP(tensor=eye16.tensor, offset=eye16.offset + b,
                          ap=[list(eye16.ap[0]), [0, 2], [0, 512]])
            nc.tensor.matmul(pzv, ccomb[:, :, j * 128:(j + 1) * 128],
                             sel, start=False, stop=True, perf_mode=DR)

        ebs = {}

        def issue_vdot(b):
            # logits + finals psum for batch b (one full PSUM bank so the
            # two lpool slots land in different zero regions):
            #   cols 0..15  logits (token-major, tile s)
            #   cols 16,17  output numerator IN1-chunks
            #   col  18     softmax denominator S (partition 0)
            psum_lt = lpool.tile([128, 512], F32, tag="lt", name=f"lt{b}")
            lts[b] = psum_lt
            u_sb = us.pop(b)
            for s in range(N // 128):     # V-dot, 16 token tiles
                for j in range(4):
                    g, o = j * 4 + s // 4, (s % 4) * 128
                    nc.tensor.matmul(
                        psum_lt[:, s:s + 1],
                        u_sb[:, g, o:o + 128],
                        vbt[:, j:j + 1],
                        start=(j == 0), stop=(j == 3))
            e_sb = epool.tile([128, N // 128], BF16, tag="e")
            nc.scalar.activation(e_sb, psum_lt[:, 0:N // 128], Exp)
            ebs[b] = e_sb

        def issue_finals(b):
            psum_lt, e_sb, en_sb = lts.pop(b), ebs.pop(b), ens.pop(b)
            # finals: three SEQUENTIAL accumulation groups (interleaved
            # groups in one psum zero-region are illegal)
            for s in range(N // 128):
                nc.tensor.matmul(psum_lt[:, 16:17], en_sb[:, s, 0:128],
                                 e_sb[:, s:s + 1],
                                 start=(s == 0), stop=(s == N // 128 - 1))
            for s in range(N // 128):
                nc.tensor.matmul(psum_lt[:, 17:18], en_sb[:, s, 128:256],
                                 e_sb[:, s:s + 1],
                                 start=(s == 0), stop=(s == N // 128 - 1))
            for s in range(N // 128):
                nc.tensor.matmul(psum_lt[0:1, 18:19], ones_col,
                                 e_sb[:, s:s + 1],
                                 start=(s == 0), stop=(s == N // 128 - 1))
            nc.vector.tensor_copy(osb[:, 0:2, b], psum_lt[:, 16:18])
            nc.vector.tensor_copy(osb[0:1, 2, b:b + 1], psum_lt[0:1, 18:19])
            nc.sync.dma_start(
                out=bass.AP(tensor=onum, offset=b,
                            ap=[[3 * bc, 128], [bc, 3], [1, 1]]),
                in_=osb[:, :, b])

        for b in range(bc):
            t0 = b * N
            q1_sb = qpool.tile([128, 2, N], F8E4, tag="q1")
            q2_sb = qpool.tile([128, 2, N], F8E4, tag="q2")
            nc.sync.dma_start(
                out=q1_sb.bitcast(U8),
                in_=bass.AP(tensor=q1d, offset=t0,
                            ap=[[2 * tok, 128], [tok, 2], [1, N]]))
            nc.sync.dma_start(
                out=q2_sb.bitcast(U8),
                in_=bass.AP(tensor=q2d, offset=t0,
                            ap=[[2 * tok, 128], [tok, 2], [1, N]]))
            en_sb = enp.tile([128, N // 128, IN1], BF16, tag="en")
            nc.gpsimd.dma_start(
                out=en_sb.bitcast(U16),
                in_=encn[t0:t0 + N, :].rearrange("(s p) c -> p s c", p=128))
            ens[b] = en_sb
            u_sb = upool.tile([128, 16, 512], BF16, tag="u")
            us[b] = u_sb
            # 16 slices g = (chunk j = g//4, token quarter v = g%4),
            # tanh'd 3 slices at a time (last instr covers just 1)
            for g0 in range(0, 16, 3):
                k = min(3, 16 - g0)
                if k == 3:
                    pz = zpool.tile([128, 3 * 512], F32, tag="zb", bufs=2,
                                    name=f"zb{b}_{g0}")
                else:
                    pz = zpool.tile([128, 512], F32, tag="z1", bufs=1,
                                    name=f"z1{b}_{g0}")
                for i in range(k):
                    g = g0 + i
                    j, v = g // 4, g % 4
                    mains(pz[:, i * 512:(i + 1) * 512],
                          q1_sb[:, :, v * 512:(v + 1) * 512],
                          q2_sb[:, :, v * 512:(v + 1) * 512], j, b)
                nc.scalar.activation(
                    u_sb[:, g0:g0 + k, :].rearrange("p g t -> p (g t)"),
                    pz[:, 0:k * 512], Tanh)
                # interleave last batch's V-dot/exp after block 0 and its
                # finals after block 1, so the in-order PE/ACT queues
                # never park on them
                if b > 0 and g0 == 0:
                    issue_vdot(b - 1)
                if b > 0 and g0 == 3:
                    issue_finals(b - 1)
        issue_vdot(bc - 1)
        issue_finals(bc - 1)



    return nc


def _bits16(x):
    return np.ascontiguousarray(x.astype(ml_dtypes.bfloat16)).view(np.uint16)


def kernel(**inputs):
    global LAST_RUNNER, _CACHED_NC
    enc = np.asarray(inputs["enc_outputs"], dtype=np.float32)   # [B, N, IN1]
    h0 = np.asarray(inputs["h0"], dtype=np.float32)             # [B, IN2]
    W1 = np.asarray(inputs["W1"], dtype=np.float32)             # [H, IN1]
    W2 = np.asarray(inputs["W2"], dtype=np.float32)             # [H, IN2]
    b2 = np.asarray(inputs["b2"], dtype=np.float32)             # [H]
    V = np.asarray(inputs["V"], dtype=np.float32)               # [H, 1]

    E4, E5 = ml_dtypes.float8_e4m3, ml_dtypes.float8_e5m2

    # W1 fp8 + residual, DoubleRow layout [p, plane, chunk, h]:
    # value = W1[chunk*128 + h, plane*128 + p]
    p1 = W1.astype(E4)
    p2 = (W1 - p1.astype(np.float32)).astype(E5)
    def w1_dr(q):
        # [H, IN1] -> [IN1-part 128, plane 2, chunk 4, h 128]
        a = q.reshape(4, 128, 2, 128).transpose(3, 2, 0, 1)
        return np.ascontiguousarray(a).view(np.uint8).reshape(128, 2 * 4 * 128)
    w1p1 = w1_dr(p1)
    w1p2 = w1_dr(p2)

    w2t = _bits16(W2.T)                                         # [IN2, H]
    b2r = _bits16(b2.reshape(1, H))
    vbr = _bits16(V.reshape(4, 128).T)                          # [128, 4]
    eyed = np.ascontiguousarray(np.eye(BC, dtype=E4)).view(np.uint8)

    in_maps = []
    for c in range(NCORES):
        enc_c = enc[c * BC:(c + 1) * BC].reshape(TOK, IN1)      # [tok, 256]
        q1 = enc_c.astype(E4)
        q2 = (enc_c - q1.astype(np.float32)).astype(E4)
        def enc_dr(q):
            # [tok, IN1] -> [p 128, plane 2, tok]
            a = q.view(np.uint8).T.reshape(2, 128, TOK).transpose(1, 0, 2)
            return np.ascontiguousarray(a).reshape(128, 2 * TOK)
        h0t = _bits16(h0[c * BC:(c + 1) * BC].T)                # [IN2, 16]
        in_maps.append({
            "q1d": enc_dr(q1), "q2d": enc_dr(q2),
            "w1p1": w1p1, "w1p2": w1p2,
            "encn": _bits16(enc_c), "w2t": w2t, "h0t": h0t,
            "b2r": b2r, "vbr": vbr, "eyed": eyed,
        })

    if _CACHED_NC is None:
        _CACHED_NC = build_nc()
    nc = _CACHED_NC

    runner = Runner(nc, in_maps)
    LAST_RUNNER = runner
    results = runner.outputs(runner.run())
    outs = []
    for c in range(NCORES):
        onum = results[c]["onum"].reshape(128, 3, BC)
        num = onum[:, 0:2, :].transpose(2, 1, 0).reshape(BC, IN1)
        s = onum[0, 2, :]                                       # [bc]
        outs.append(num / s[:, None])
    return np.concatenate(outs, axis=0).astype(np.float32)
